# revision 1
# baseline (speedup 1.0000x reference)
"""HaarDeconv2D (vertical, 2x1, stride (2,1)) Trainium2 kernel.

Math: with L=[0.5,0.5], D=[0.5,-0.5],
  even = 0.5*(low+detail) + 0.5*(low-detail) = low_pass
  odd  = 0.5*(low+detail) - 0.5*(low-detail) = detail
so the output is exactly a row-interleave of the two inputs along H.
This is pure data movement: two strided DRAM->DRAM DMAs per core, no
compute engines at all. Sharded over batch (16 -> 2 per core, 8 cores).
"""

import numpy as np

_N_CORES = 8
_B, _C, _H, _W = 16, 3, 512, 1024
_BS = _B // _N_CORES  # batches per core

_nc_cache = None


def _build():
    global _nc_cache
    if _nc_cache is not None:
        return _nc_cache
    import concourse.bass as bass
    import concourse.mybir as mybir

    nc = bass.Bass()
    low = nc.dram_tensor(
        "low", [_BS, _C, _H, _W], mybir.dt.float32, kind="ExternalInput"
    )
    det = nc.dram_tensor(
        "det", [_BS, _C, _H, _W], mybir.dt.float32, kind="ExternalInput"
    )
    out = nc.dram_tensor(
        "out", [_BS, _C, 2 * _H, _W], mybir.dt.float32, kind="ExternalOutput"
    )
    # out viewed as [b, c, h, 2, w]: slot 0 rows come from low, slot 1 from det
    ov = out[:].rearrange("b c (h two) w -> b c h two w", two=2)

    with nc.Block() as block, nc.semaphore("dma_sem") as dma_sem:

        @block.sync
        def _(sync):
            sync.dma_start(out=ov[:, :, :, 0, :], in_=low[:]).then_inc(dma_sem, 16)
            sync.dma_start(out=ov[:, :, :, 1, :], in_=det[:]).then_inc(dma_sem, 16)
            sync.wait_ge(dma_sem, 32)

    _nc_cache = nc
    return nc


def kernel(low_pass, detail):
    from concourse.bass_utils import run_bass_kernel_spmd

    low_pass = np.ascontiguousarray(np.asarray(low_pass, dtype=np.float32))
    detail = np.ascontiguousarray(np.asarray(detail, dtype=np.float32))
    nc = _build()
    in_maps = [
        {
            "low": low_pass[i * _BS : (i + 1) * _BS],
            "det": detail[i * _BS : (i + 1) * _BS],
        }
        for i in range(_N_CORES)
    ]
    r = run_bass_kernel_spmd(nc, in_maps, core_ids=list(range(_N_CORES)))
    return np.concatenate([res["out"] for res in r.results], axis=0)
